# revision 41
# baseline (speedup 1.0000x reference)
import sys

sys.path.insert(0, "/opt/trn_rl_repo")

import os
import numpy as np

PROBE = os.environ.get("KPROBE", "")

G, E, N, H = 8, 8192, 512, 32
NP1 = N + 1          # 513
T = N * N            # 262144 tokens per graph
V = H * NP1 * NP1    # flat output elements per graph

NBLK = 32            # main-pass blocks
RPB = N // NBLK      # 16 output rows per block
CPB = RPB * 512      # 8192 tokens per block
NA3 = 4              # resident attn3 tiles
VS = NBLK * 128 * (RPB // 4) * 512   # scrambled main output elements


# ----------------------------------------------------------------- device code
def build(nc, outs, ins):
    from contextlib import ExitStack

    import concourse.tile as tile
    from concourse import mybir

    f32 = mybir.dt.float32
    f16 = mybir.dt.float16
    bf16 = mybir.dt.bfloat16
    Relu = mybir.ActivationFunctionType.Relu

    out_scr = outs["out_scr"]         # [VS] f32, scrambled SBUF-order blocks
    out_str = outs["out_str"]         # [2, 32, NP1] f32 (row0 strip; col0 strip)
    xcat = ins["xcat"]                # [NBLK*128*CPB] f16 (hi rows 0-56, lo-bits 64-120)
    attn12 = ins["attn12"]            # [12, T//4] f16 (attn rows grouped by 4)
    strips = ins["strips"]            # [2, NP1] f32
    w1hcat2 = ins["w1hcat2"]          # [128, 64] f16 (w1h at rows 0-56 and 64-120)
    w1lb = ins["w1lb"]                # [64, 64] bf16 (rows 57-63 zero)
    w2bdf = ins["w2bdf"]              # [128, 64] f32 block-diag (2-chunk packed)
    ones12 = ins["ones12"]            # [12, 128] f16 (block-diag 1,1,2^-12)
    extw = ins["extw"]                # [2, 32] f32 (ones row; virt row)

    xv = xcat.rearrange("(b p q) -> b p q", p=128, q=CPB)
    ov = out_scr.rearrange("(b p q) -> b p q", p=128, q=(RPB // 4) * 512)

    with tile.TileContext(nc) as tc, ExitStack() as ctx:
        cst = ctx.enter_context(tc.tile_pool(name="cst", bufs=1))
        xpool = ctx.enter_context(tc.tile_pool(name="xpool", bufs=4))
        hpool = ctx.enter_context(tc.tile_pool(name="hpool", bufs=6))
        opool = ctx.enter_context(tc.tile_pool(name="opool", bufs=4))
        ps_l1 = ctx.enter_context(tc.tile_pool(name="ps_l1", bufs=4, space="PSUM"))
        ps_l2 = ctx.enter_context(tc.tile_pool(name="ps_l2", bufs=3, space="PSUM"))

        # ---- constants
        w1h_s = cst.tile([128, 64], f16)
        nc.sync.dma_start(out=w1h_s[:], in_=w1hcat2[:])
        w1lb_s = cst.tile([64, 64], bf16)
        nc.sync.dma_start(out=w1lb_s[:], in_=w1lb[:])
        w2bdf_s = cst.tile([128, 64], f32)
        nc.sync.dma_start(out=w2bdf_s[:], in_=w2bdf[:])
        ones12_s = cst.tile([12, 128], f16)
        nc.sync.dma_start(out=ones12_s[:], in_=ones12[:])
        extw_s = cst.tile([2, 32], f32)
        nc.sync.dma_start(out=extw_s[:], in_=extw[:])

        apool = ctx.enter_context(tc.tile_pool(name="apool", bufs=3))

        # ---- main pass: RPB output rows per block
        for blk in range(NBLK):
            xc = xpool.tile([128, CPB], f16, tag="xc")
            nc.sync.dma_start(out=xc[:], in_=xv[blk])
            a12_t = apool.tile([12, CPB // 4], f16, tag="a12")
            nc.gpsimd.dma_start(
                out=a12_t[:], in_=attn12[:, blk * (CPB // 4):(blk + 1) * (CPB // 4)])

            osb = opool.tile([128, (RPB // 4) * 512], f32, tag="osb")
            for sub in range(RPB // 4):
                sb = sub * 2048          # token offset of this 4-row group
                ps1a = ps_l1.tile([128, 512], f32, tag="ps1")
                ps1b = ps_l1.tile([128, 512], f32, tag="ps1")
                # batch same-weight matmuls to minimize PE weight swaps while
                # keeping at most one open accumulation group per PSUM bank
                for cp in (0, 64):
                    pairs = [(ps1a, sb + cp * 8), (ps1b, sb + 1024 + cp * 8)]
                    for ps, c in pairs:
                        nc.tensor.matmul(out=ps[cp:cp + 64, :], lhsT=w1h_s[0:128, :],
                                         rhs=xc[0:128, c:c + 512],
                                         start=True, stop=False, tile_position=(0, cp))
                    for ps, c in pairs:
                        nc.tensor.matmul(out=ps[cp:cp + 64, :], lhsT=w1lb_s[:],
                                         rhs=xc[0:64, c:c + 512],
                                         start=False, stop=True, tile_position=(0, cp))
                hids = []
                for ps in (ps1a, ps1b):
                    hid = hpool.tile([128, 512], f32, tag="hid")
                    nc.scalar.activation(out=hid[:], in_=ps[:], func=Relu)
                    hids.append(hid)

                ps2 = ps_l2.tile([128, 512], f32, tag="ps2")
                nc.tensor.matmul(
                    out=ps2[:], lhsT=ones12_s[:],
                    rhs=a12_t[:, sub * 512:(sub + 1) * 512],
                    start=True, stop=False,
                    tile_position=(0, 0),
                )
                for pair in range(2):
                    hid = hids[pair]
                    cp = 64 * pair
                    nc.tensor.matmul(
                        out=ps2[cp:cp + 64, :], lhsT=w2bdf_s[:],
                        rhs=hid[:], start=False, stop=True,
                        skip_group_check=True, tile_position=(0, cp),
                    )
                nc.vector.tensor_copy(
                    out=osb[:, sub * 512:(sub + 1) * 512], in_=ps2[:]
                )
            if "storesync" in PROBE:
                nc.sync.dma_start(out=ov[blk], in_=osb[:])
            else:
                nc.scalar.dma_start(out=ov[blk], in_=osb[:])

        if "nostrips" in PROBE:
            dummy = opool.tile([2, 32], f32, tag="osb2")
            nc.gpsimd.memset(dummy[:], 0.0)
            nc.scalar.dma_start(out=out_str[0, 0:2, 0:32], in_=dummy[:])
            return
        # ---- row-0 / col-0 strips -> out_str
        # col 0, rows 1..512: attn[p,0] + virt
        rhs0 = cst.tile([2, 512], f32)
        nc.gpsimd.memset(rhs0[:], 1.0)
        nc.sync.dma_start(out=rhs0[0:1, :], in_=strips[1:2, 1:513])
        pc0 = ps_l2.tile([32, 512], f32, tag="ps2")
        nc.tensor.matmul(out=pc0[:], lhsT=extw_s[:], rhs=rhs0[:],
                         start=True, stop=True)
        c0sb = opool.tile([32, NP1], f32, tag="osb2")
        nc.gpsimd.memset(c0sb[:], 0.0)
        nc.vector.tensor_copy(out=c0sb[:, 1:513], in_=pc0[:])
        nc.scalar.dma_start(out=out_str[1, :, :], in_=c0sb[:])

        # row 0, cols 0..512: attn[0,q] + virt
        rhsr = cst.tile([2, NP1], f32)
        nc.gpsimd.memset(rhsr[:], 1.0)
        nc.sync.dma_start(out=rhsr[0:1, :], in_=strips[0:1, :])
        pr0 = ps_l2.tile([32, 512], f32, tag="ps2")
        nc.tensor.matmul(out=pr0[:], lhsT=extw_s[:], rhs=rhsr[:, 0:512],
                         start=True, stop=True)
        r0sb = opool.tile([32, NP1], f32, tag="osb2")
        nc.vector.tensor_copy(out=r0sb[:, 0:512], in_=pr0[:])
        pr1 = ps_l2.tile([32, 1], f32, tag="ps2", bufs=3)
        nc.tensor.matmul(out=pr1[:], lhsT=extw_s[:], rhs=rhsr[:, 512:513],
                         start=True, stop=True)
        nc.vector.tensor_copy(out=r0sb[:, 512:513], in_=pr1[:])
        nc.scalar.dma_start(out=out_str[0, :, :], in_=r0sb[:])


# ----------------------------------------------------------------- host prep
def prep_core(g, inputs):
    import ml_dtypes
    attn = np.ascontiguousarray(inputs["attn_bias"][g], np.float32)
    angle = inputs["angle"][g]
    dists = inputs["dists"][g]

    xt = np.empty((57, T), np.float32)
    xt[0:28] = angle.reshape(T, 28).T
    xt[28:56] = dists.reshape(T, 28).T
    xt[56] = 1.0

    w1cat = np.zeros((57, 64), np.float32)
    w1cat[0:28, 0:32] = inputs["ang_w1"]
    w1cat[28:56, 32:64] = inputs["md_w1"]
    w1cat[56, 0:32] = inputs["ang_b1"]
    w1cat[56, 32:64] = inputs["md_b1"]
    w2 = np.concatenate([inputs["ang_w2"], inputs["md_w2"]], 0).astype(np.float32)
    # f32 block-diagonal 2-chunk packing: [0:64, 0:32] and [64:128, 32:64]
    w2bdf = np.zeros((128, 64), np.float32)
    w2bdf[0:64, 0:32] = w2
    w2bdf[64:128, 32:64] = w2
    b2sum = (np.asarray(inputs["ang_b2"]) + np.asarray(inputs["md_b2"])).astype(np.float32)

    # x split: fp16 hi + bf16 lo, co-packed into blocked [NBLK, 128, CPB] f16
    xh = xt.astype(np.float16)
    xl16 = (xt - xh.astype(np.float32)).astype(np.float16)
    xcat = np.zeros((NBLK, 128, CPB), np.float16)
    xcat[:, 0:57, :] = xh.reshape(57, NBLK, CPB).transpose(1, 0, 2)
    xcat[:, 64:121, :] = xl16.reshape(57, NBLK, CPB).transpose(1, 0, 2)
    xcat = xcat.reshape(-1)

    w1h = w1cat.astype(np.float16)
    w1lb = np.zeros((64, 64), ml_dtypes.bfloat16)
    w1lb[0:57] = (w1cat - w1h.astype(np.float32)).astype(ml_dtypes.bfloat16)
    w1hcat2 = np.zeros((128, 64), np.float16)
    w1hcat2[0:57] = w1h
    w1hcat2[64:121] = w1h

    # attn inner block, split into fp16 hi/mid/lo*2^12 (exact to ~2^-34)
    a = attn[1:, 1:]                           # [N, N]
    hi = a.astype(np.float16)
    r1 = a - hi.astype(np.float32)
    mid = r1.astype(np.float16)
    r2 = r1 - mid.astype(np.float32)
    lo12 = (r2 * 4096.0).astype(np.float16)
    # attn12[3r+c, j, :] = comp_c[4j + r, :]  (rows grouped per 4-row psum)
    comps = np.stack([hi, mid, lo12])                  # [3, N, N]
    attn12 = np.empty((12, N // 4, 512), np.float16)
    for r_ in range(4):
        for c_ in range(3):
            attn12[3 * r_ + c_] = comps[c_, r_::4, :]
    attn12 = attn12.reshape(12, T // 4)
    ones12 = np.zeros((12, 128), np.float16)
    for r_ in range(4):
        ones12[3 * r_ + 0, 32 * r_:32 * r_ + 32] = 1.0
        ones12[3 * r_ + 1, 32 * r_:32 * r_ + 32] = 1.0
        ones12[3 * r_ + 2, 32 * r_:32 * r_ + 32] = 2.0 ** -12

    strips = np.zeros((2, NP1), np.float32)
    strips[0] = attn[0, :]
    strips[1, 1:] = attn[1:, 0]

    ones3 = np.zeros((3, 32), np.float16)
    ones3[0] = 1.0
    ones3[1] = 1.0
    ones3[2] = 2.0 ** -12

    extw = np.zeros((2, 32), np.float32)
    extw[0] = 1.0
    extw[1] = np.asarray(inputs["virt"], np.float32).reshape(32)

    m = dict(xcat=xcat, attn12=attn12, strips=strips, w1hcat2=w1hcat2,
             w1lb=w1lb, w2bdf=w2bdf, ones12=ones12, extw=extw)
    return m, b2sum


def edge_emb_host(g, inputs):
    """Edge embeddings + flat scatter indices, computed exactly as reference."""
    ef = np.asarray(inputs["edge_feat"][g])
    ei = np.asarray(inputs["edge_index"][g]).astype(np.int64)
    mask = np.asarray(inputs["edge_mask"][g]).astype(bool)
    nlig = max(int(inputs["num_ligand_atoms"][g]), 1)

    t0 = ef[:, 0].astype(np.int32)
    t1 = ef[:, 1].astype(np.int32)
    t2 = ef[:, 2].astype(np.int32)
    d = ef[:, 3:4].astype(np.float32)          # [E, 1]
    src, tgt = ei[0], ei[1]
    src_l = (src > 0) & (src < nlig)
    tgt_l = (tgt > 0) & (tgt < nlig)

    h1 = np.maximum(d @ np.asarray(inputs["dist_w1"], np.float32)
                    + np.asarray(inputs["dist_b1"], np.float32), 0.0)
    demb = h1 @ np.asarray(inputs["dist_w2"], np.float32) \
        + np.asarray(inputs["dist_b2"], np.float32)       # [E, 32]

    sidx = np.clip(t0 * 4 + t1 * 2 + t2, 0, 19)
    structural = np.asarray(inputs["struct_emb"], np.float32)[sidx]
    pidx = np.clip(t1, 0, 14)
    plip = np.where(
        (src_l & tgt_l)[:, None], np.asarray(inputs["plip_lig"], np.float32)[pidx],
        np.where((~src_l & ~tgt_l)[:, None],
                 np.asarray(inputs["plip_prot"], np.float32)[pidx],
                 np.asarray(inputs["plip_inter"], np.float32)[pidx]))
    emb = np.where((t0 <= 1)[:, None], structural,
                   np.where((t0 == 5)[:, None], plip, 0.0)) + demb
    emb = emb * mask[:, None].astype(np.float32)          # [E, 32]

    cell = (src + 1) * NP1 + (tgt + 1)                    # [E]
    h_off = np.arange(H, dtype=np.int64) * (NP1 * NP1)
    idx = cell[:, None] + h_off[None, :]                  # [E, 32]
    return emb, idx


_IN_SPECS = [
    ("xcat", (NBLK * 128 * CPB,), "float16"),
    ("attn12", (12, T // 4), "float16"),
    ("strips", (2, NP1), "float32"),
    ("w1hcat2", (128, 64), "float16"),
    ("w1lb", (64, 64), "bfloat16"),
    ("w2bdf", (128, 64), "float32"),
    ("ones12", (12, 128), "float16"),
    ("extw", (2, 32), "float32"),
]


def _patch_ldw_opt():
    """Enable walrus LDWEIGHTS dedup (concourse pins it off)."""
    from concourse import bass_utils as _bu

    if getattr(_bu, "_ldw_opt_patched", False):
        return
    _orig = _bu.run_command

    def _patched(cmd, cwd=None, **kw):
        cmd = [c.replace("--enable-ldw-opt=false", "--enable-ldw-opt=true")
               if isinstance(c, str) and False else c for c in cmd]
        return _orig(cmd, cwd=cwd, **kw)

    _bu.run_command = _patched
    _bu._ldw_opt_patched = True


def _build_nc():
    from concourse import bacc, mybir

    _patch_ldw_opt()

    nc = bacc.Bacc(
        "TRN2",
        target_bir_lowering=False,
        debug=False,
        enable_asserts=False,
        num_devices=8,
    )
    ins = {}
    for name, shape, dt_name in _IN_SPECS:
        h = nc.dram_tensor(name, list(shape), getattr(mybir.dt, dt_name),
                           kind="ExternalInput")
        ins[name] = h[:]
    scr_h = nc.dram_tensor("out_scr", [VS], mybir.dt.float32,
                           kind="ExternalOutput")
    str_h = nc.dram_tensor("out_str", [2, 32, NP1], mybir.dt.float32,
                           kind="ExternalOutput")
    build(nc, {"out_scr": scr_h[:], "out_str": str_h[:]}, ins)
    nc.compile()
    return nc


def kernel(_trace=False, **inputs):
    from concourse.bass_utils import run_bass_kernel_spmd

    in_maps = []
    b2sums = []
    edges = []
    for g in range(G):
        m, b2sum = prep_core(g, inputs)
        in_maps.append(m)
        b2sums.append(b2sum)
        edges.append(edge_emb_host(g, inputs))

    nc = _build_nc()
    res = run_bass_kernel_spmd(nc, in_maps, core_ids=list(range(G)), trace=_trace)
    if _trace:
        print("HW exec time:", res.exec_time_ns, "ns  (mean:", res.mean_exec_time_ns,
              "ns, slowest core:", res.max_exec_time_core_id, ")")
        if res.instructions_and_trace:
            print("trace:", res.instructions_and_trace[1])
    outs = []
    for g, r in enumerate(res.results):
        flat = np.zeros(V, np.float32)
        # unscramble: [NBLK, 2?, (r h), (sub q)] -> [h, p, q]
        scr = r["out_scr"].reshape(NBLK, 4, 32, RPB // 4, 512)
        # dims: (blk, r, h, sub, q); p = 1 + blk*RPB + sub*4 + r
        inner = scr.transpose(2, 0, 3, 1, 4).reshape(H, N, 512)
        v3 = flat.reshape(H, NP1, NP1)
        v3[:, 1:, 1:] = inner
        stp = r["out_str"]
        v3[:, 0, :] = stp[0]
        v3[:, 1:, 0] = stp[1][:, 1:]
        emb, idx = edges[g]
        np.add.at(flat, idx.ravel(), emb.ravel())
        outs.append(flat.reshape(H, NP1, NP1))
    out = np.stack(outs)
    b2s = np.stack(b2sums)  # [G, 32]
    if np.any(b2s != 0):
        out[:, :, 1:, 1:] += b2s[:, :, None, None]
    return out.astype(np.float32)
